# revision 13
# baseline (speedup 1.0000x reference)
"""Paged-attention decode kernel for 8 TRN2 NeuronCores (8-bit KV stream).

Problem: B=16 decode sequences, H=16 heads, D=128 head dim, paged KV cache
(2048 blocks x 16 tokens), context S=2048 per sequence.

Sharding: data-parallel over sequences -- 2 sequences per core, no
collectives.  The host applies the KV-cache scatter (slot_mapping), the
paged gather (block_tables), and 8-bit encoding while laying out per-core
shards; the device does the full masked single-token attention.

8-bit wire format (halves the 33.6MB/core bf16 KV stream to 16.8MB):
  K: float8 e3m4, consumed DIRECTLY by the PE as the stationary QK operand
     (mixed fp8xbf16 matmul decodes exactly, subnormals included).
  V: int8 with a per-token fp32 scale, dequantized to bf16 on the DVE
     (tensor_scalar hits 2X mode even with int8 input: ~1.28us per tile),
     then consumed as the moving PV operand.

Device math per (seq, 128-token tile), fp32 accumulate:
  scores[s,h] = sum_d K8[d,s] * (q[d,h]*SCALE)   16 PE pairs (~40ns each)
  e[s,h]      = exp(scores + mask_bias[s])       ScalarE, bias trick
  vdq[s,:]    = bf16(V8[s,:] * vscale[s])        DVE
  o_num[h',:] += e.T @ vdq                       4 PE matmuls, PSUM accum
  denom[h']   += e.T @ ones                      1 PE matmul
  out[h', :]  = o_num[h', :] / denom[h']         final normalize, bf16

The PE is the pacing engine (~1.8us/tile: QK 0.64 + PV/sums 1.14).  Both
sequences run as one flat 32-tile stream; all chunk DMAs are issued
upfront so the two HWDGE rings (K on sync, V on scalar) self-pace on
buffer release and never starve the PE.  QK+exp+dequant run two tiles
ahead of PV.  Expected rel err ~1.7e-2 (numpy-exact sim of this pipeline;
K-e3m4 rounding dominates).
"""

import numpy as np
import ml_dtypes

from concourse import bass, bacc, mybir, tile
from concourse.bass_utils import run_bass_kernel_spmd

# Problem constants (hardcoded per the grading contract).
B = 16          # total sequences
H = 16          # heads
D = 128         # head dim
BLOCK = 16      # tokens per cache block
BPS = 128       # blocks per sequence
NB = B * BPS    # total cache blocks
S = BPS * BLOCK # max context per sequence (2048)
SCALE = 0.08838834764831845

N_CORES = 8
B2 = B // N_CORES             # sequences per core (2)
T = S // 128                  # 128-token tiles per sequence (16)
CHUNKS = (1, 2, 3, 4, 3, 2, 1)  # per-seq KV stream chunking (tiles per DMA)
assert sum(CHUNKS) == T
HD = H * D                    # 2048: free width of one V tile / K tile

F32 = mybir.dt.float32
BF16 = mybir.dt.bfloat16
E3 = mybir.dt.float8e3
I8 = mybir.dt.int8
NP_BF16 = ml_dtypes.bfloat16
NP_E3 = ml_dtypes.float8_e3m4

MASK_NEG = -30000.0  # exp(x + MASK_NEG) == 0 in fp32 for any |x| < ~100


def build_nc(b2=B2, chunks=CHUNKS):
    """Build the per-core Bass graph (SPMD: same graph on all 8 cores)."""
    t_tiles = sum(chunks)
    sizes = sorted(set(chunks))
    nc = bacc.Bacc("TRN2", target_bir_lowering=False, debug=False)

    n_of = {sz: sum(1 for c in chunks if c == sz) for sz in sizes}
    # K chunk: [d=128, (tile, h, s)] fp8e3; V chunk: [s=128, (tile, h, d)] int8
    kk_p = {sz: nc.declare_dram_parameter(
        f"kk{sz}", [b2, n_of[sz], 128, sz * HD], E3, isOutput=False)
        for sz in sizes}
    vv_p = {sz: nc.declare_dram_parameter(
        f"vv{sz}", [b2, n_of[sz], 128, sz * HD], I8, isOutput=False)
        for sz in sizes}
    qt = nc.declare_dram_parameter("qt", [b2, 128, H], BF16, isOutput=False)
    bias = nc.declare_dram_parameter("bias", [b2, 128, t_tiles], F32,
                                     isOutput=False)
    vsc = nc.declare_dram_parameter("vsc", [b2, 128, t_tiles], F32,
                                    isOutput=False)
    # PV numerator in all-heads layout [b, h', (h, d)], already normalized;
    # the host extracts the h'==h diagonal
    out = nc.declare_dram_parameter("out", [b2, H, HD], BF16, isOutput=True)

    # global chunk list across both sequences
    gchunks = []       # (b, sz, param_idx, first_local_tile)
    for b in range(b2):
        seen = {sz: 0 for sz in sizes}
        t0 = 0
        for sz in chunks:
            gchunks.append((b, sz, seen[sz], t0))
            seen[sz] += 1
            t0 += sz
    tile2chunk = []    # global tile -> global chunk
    for ci, (_, sz, _, _) in enumerate(gchunks):
        tile2chunk += [ci] * sz
    n_gtiles = b2 * t_tiles

    with tile.TileContext(nc) as tc:
        with (
            tc.tile_pool(name="const", bufs=1) as cpool,
            tc.tile_pool(name="kpool", bufs=2) as kpool,
            tc.tile_pool(name="vpool", bufs=2) as vpool,
            tc.tile_pool(name="vdq", bufs=4) as dqpool,
            tc.tile_pool(name="small", bufs=2) as spool,
            tc.tile_pool(name="pscore", bufs=3,
                         space=bass.MemorySpace.PSUM) as pscore,
            tc.tile_pool(name="pacc", bufs=1,
                         space=bass.MemorySpace.PSUM) as pacc,
        ):
            ones_t = cpool.tile([128, 1], BF16, tag="ones")
            nc.gpsimd.memset(ones_t[:], 1.0)

            # small per-seq inputs: ship before the chunk stream
            qt_sb, bias_sb, vsc_sb = {}, {}, {}
            for b in range(b2):
                qt_sb[b] = spool.tile([128, H], BF16, tag="qt_sb",
                                      name=f"qt_sb{b}")
                nc.scalar.dma_start(out=qt_sb[b][:], in_=qt[b])
                bias_sb[b] = spool.tile([128, t_tiles], F32, tag="bias_sb",
                                        name=f"bias_sb{b}")
                nc.scalar.dma_start(out=bias_sb[b][:], in_=bias[b])
                vsc_sb[b] = spool.tile([128, t_tiles], F32, tag="vsc_sb",
                                       name=f"vsc_sb{b}")
                nc.scalar.dma_start(out=vsc_sb[b][:], in_=vsc[b])

            hw = HD // 2
            # one accumulator set shared by both sequences (PSUM is 8 banks);
            # seq1's first accumulate waits on seq0's finalize reads
            ps_o_lo = pacc.tile([H, hw], F32, tag="ps_o_lo")
            ps_o_hi = pacc.tile([H, hw], F32, tag="ps_o_hi")
            ps_sums = pacc.tile([H, 1], F32, tag="ps_sums")

            kk_tiles, vv_tiles = {}, {}
            issued = [0]   # chunks issued so far

            def issue_up_to(ci_max):
                """Lazily issue chunk DMAs so ~2 chunks stay in flight."""
                while issued[0] <= min(ci_max, len(gchunks) - 1):
                    ci = issued[0]
                    b, sz, pi, _ = gchunks[ci]
                    nbuf = 3 if sz >= 3 else 2
                    kc = kpool.tile([128, sz * HD], E3, tag=f"kk{sz}",
                                    bufs=nbuf, name=f"kc{ci}")
                    nc.sync.dma_start(out=kc[:], in_=kk_p[sz][b, pi])
                    vc = vpool.tile([128, sz * HD], I8, tag=f"vv{sz}",
                                    bufs=nbuf, name=f"vc{ci}")
                    nc.scalar.dma_start(out=vc[:], in_=vv_p[sz][b, pi])
                    kk_tiles[ci], vv_tiles[ci] = kc, vc
                    issued[0] += 1

            def qk_exp(g):
                """QK (16 PE pairs) + exp -> e_t [128, H] bf16."""
                ci = tile2chunk[g]
                b, sz, _, ct0 = gchunks[ci]
                t = g - b * t_tiles               # local tile index
                kc = kk_tiles[ci]
                ps_sc = pscore.tile([128, H], F32, tag="ps_sc", bufs=3)
                base = (t - ct0) * HD
                for hh in range(H):
                    o0 = base + hh * 128
                    nc.tensor.matmul(
                        ps_sc[:, hh:hh + 1],
                        kc[:, o0:o0 + 128],
                        qt_sb[b][:, hh:hh + 1],
                        start=True, stop=True,
                        skip_group_check=True,
                    )
                e_t = spool.tile([128, H], BF16, tag="e_t", bufs=4)
                nc.scalar.activation(
                    e_t[:], ps_sc[:],
                    mybir.ActivationFunctionType.Exp,
                    bias=bias_sb[b][:, t:t + 1], scale=1.0,
                )
                return e_t

            def dequant(g):
                """V int8 -> bf16 with per-token scale (DVE)."""
                ci = tile2chunk[g]
                b, sz, _, ct0 = gchunks[ci]
                t = g - b * t_tiles
                vc = vv_tiles[ci]
                o0 = (t - ct0) * HD
                vq = dqpool.tile([128, HD], BF16, tag="vdq", bufs=4)
                nc.vector.tensor_scalar_mul(vq[:], vc[:, o0:o0 + HD],
                                            vsc_sb[b][:, t:t + 1])
                return vq

            def finalize(b):
                recip = spool.tile([H, 1], F32, tag="recip")
                nc.vector.reciprocal(recip[:], ps_sums[:])
                o_lo = spool.tile([H, hw], BF16, tag="o_lo")
                o_hi = spool.tile([H, hw], BF16, tag="o_hi")
                nc.scalar.mul(o_lo[:], ps_o_lo[:], recip[:])
                nc.vector.tensor_scalar_mul(o_hi[:], ps_o_hi[:], recip[:])
                if b == b2 - 1:
                    nc.sync.dma_start(out=out[b][:, 0:hw], in_=o_lo[:])
                    nc.scalar.dma_start(out=out[b][:, hw:], in_=o_hi[:])
                else:
                    nc.gpsimd.dma_start(out=out[b][:, 0:hw], in_=o_lo[:])
                    nc.gpsimd.dma_start(out=out[b][:, hw:], in_=o_hi[:])

            issue_up_to(1)
            pend = [(qk_exp(0), dequant(0))]
            issue_up_to(tile2chunk[1] + 2)
            pend.append((qk_exp(1), dequant(1)))
            for g in range(n_gtiles):
                if g + 2 < n_gtiles:
                    # keep ~2 chunks of DMA in flight past the one QK reads
                    issue_up_to(tile2chunk[g + 2] + 2)
                    pend.append((qk_exp(g + 2), dequant(g + 2)))
                e_t, vq = pend.pop(0)

                b = g // t_tiles
                t = g - b * t_tiles
                first = t == 0
                last = t == t_tiles - 1
                nc.tensor.matmul(ps_sums[:], e_t[:], ones_t[:],
                                 start=first, stop=last,
                                 skip_group_check=True)
                for n in range(4):
                    o0 = n * 512
                    dst = ps_o_lo if n < 2 else ps_o_hi
                    nc.tensor.matmul(
                        dst[:, (n % 2) * 512:(n % 2 + 1) * 512],
                        e_t[:],
                        vq[:, o0:o0 + 512],
                        start=first, stop=last,
                        skip_group_check=True,
                    )
                if last:
                    finalize(b)

    nc.compile()
    return nc


def prep_in_maps(q, k, v, k_cache, v_cache, block_tables, slot_mapping,
                 context_lens):
    """Host-side scatter + paged gather + 8-bit encode + per-core shards."""
    q = np.asarray(q, np.float32)
    k = np.asarray(k, np.float32)
    v = np.asarray(v, np.float32)
    k_cache = np.asarray(k_cache, np.float32)
    v_cache = np.asarray(v_cache, np.float32)
    block_tables = np.asarray(block_tables, np.int32)
    slot_mapping = np.asarray(slot_mapping, np.int64)
    context_lens = np.asarray(context_lens, np.int32)

    nb, block_size, h, d = k_cache.shape
    kc = k_cache.reshape(nb * block_size, h, d).copy()
    kc[slot_mapping] = k
    vc = v_cache.reshape(nb * block_size, h, d).copy()
    vc[slot_mapping] = v
    k_seq = kc.reshape(nb, block_size, h, d)[block_tables].reshape(B, S, h, d)
    v_seq = vc.reshape(nb, block_size, h, d)[block_tables].reshape(B, S, h, d)

    # K: [B,S,H,D] -> [B, D, T, H, 128s] fp8 e3m4
    kt = np.ascontiguousarray(
        k_seq.reshape(B, T, 128, H, D).transpose(0, 4, 1, 3, 2)
    ).astype(NP_E3).reshape(B, 128, T * HD)

    # V: int8 with per-token scale. st: [B, S]
    st = np.abs(v_seq).reshape(B, S, HD).max(axis=2) / 127.0
    v8 = np.clip(np.round(v_seq.reshape(B, S, HD) / st[:, :, None]),
                 -127, 127).astype(np.int8)
    # [B, S, HD] -> [B, 128s, T, HD]
    v8t = np.ascontiguousarray(
        v8.reshape(B, T, 128, HD).transpose(0, 2, 1, 3))

    sizes = sorted(set(CHUNKS))
    kk_parts = {sz: [] for sz in sizes}
    v_parts = {sz: [] for sz in sizes}
    t0 = 0
    for sz in CHUNKS:
        kk_parts[sz].append(
            kt[:, None, :, t0 * HD:(t0 + sz) * HD])
        v_parts[sz].append(np.ascontiguousarray(
            v8t[:, :, t0:t0 + sz]).reshape(B, 1, 128, sz * HD))
        t0 += sz
    kk_host = {sz: np.ascontiguousarray(np.concatenate(kk_parts[sz], axis=1))
               for sz in sizes}
    v_host = {sz: np.concatenate(v_parts[sz], axis=1) for sz in sizes}

    qs = (q * SCALE).astype(NP_BF16)
    qt_host = np.ascontiguousarray(qs.transpose(0, 2, 1))  # [B, D, H]
    s_idx = np.arange(S, dtype=np.int64)
    m = np.where(s_idx[None, :] < context_lens[:, None].astype(np.int64),
                 0.0, MASK_NEG).astype(np.float32)
    bias_host = np.ascontiguousarray(m.reshape(B, T, 128).transpose(0, 2, 1))
    vsc_host = np.ascontiguousarray(
        st.astype(np.float32).reshape(B, T, 128).transpose(0, 2, 1))

    in_maps = []
    for i in range(N_CORES):
        lo, hi = i * B2, (i + 1) * B2
        im = {"qt": np.ascontiguousarray(qt_host[lo:hi]),
              "bias": np.ascontiguousarray(bias_host[lo:hi]),
              "vsc": np.ascontiguousarray(vsc_host[lo:hi])}
        for sz in sizes:
            im[f"kk{sz}"] = np.ascontiguousarray(kk_host[sz][lo:hi])
            im[f"vv{sz}"] = np.ascontiguousarray(v_host[sz][lo:hi])
        in_maps.append(im)
    return in_maps


_NC = None


def _get_nc():
    global _NC
    if _NC is None:
        _NC = build_nc()
    return _NC


def run(inputs, trace=False, **spmd_kwargs):
    """Run on hardware; returns (full_output, BassKernelResults)."""
    nc = _get_nc()
    in_maps = prep_in_maps(**inputs)
    res = run_bass_kernel_spmd(nc, in_maps, core_ids=list(range(N_CORES)),
                               trace=trace, **spmd_kwargs)
    out_full = np.concatenate([res.results[i]["out"] for i in range(N_CORES)],
                              axis=0).astype(np.float32)
    # extract the h'==h diagonal: [B, H, H*D] -> [B, H, D]
    hh = np.arange(H)
    out = out_full.reshape(B, H, H, D)[:, hh, hh, :]
    return np.ascontiguousarray(out), res


def kernel(**inputs) -> np.ndarray:
    out, _ = run(inputs, trace=False)
    return out


# revision 14
# speedup vs baseline: 1.0834x; 1.0834x over previous
"""Paged-attention decode kernel for 8 TRN2 NeuronCores (8-bit KV stream).

Problem: B=16 decode sequences, H=16 heads, D=128 head dim, paged KV cache
(2048 blocks x 16 tokens), context S=2048 per sequence.

Sharding: data-parallel over sequences -- 2 sequences per core, no
collectives.  The host applies the KV-cache scatter (slot_mapping), the
paged gather (block_tables), and 8-bit encoding while laying out per-core
shards; the device does the full masked single-token attention.

8-bit wire format (halves the 33.6MB/core bf16 KV stream to 16.8MB):
  K: float8 e3m4, consumed DIRECTLY by the PE as the stationary QK operand
     (mixed fp8xbf16 matmul decodes exactly, subnormals included).
  V: int8 with a per-token fp32 scale, dequantized to bf16 on the DVE
     (tensor_scalar hits 2X mode even with int8 input: ~1.28us per tile),
     then consumed as the moving PV operand.

Device math per (seq, 128-token tile), fp32 accumulate:
  scores[s,h] = sum_d K8[d,s] * (q[d,h]*SCALE)   16 PE pairs (~40ns each)
  e[s,h]      = exp(scores + mask_bias[s])       ScalarE, bias trick
  vdq[s,:]    = bf16(V8[s,:] * vscale[s])        DVE
  o_num[h',:] += e.T @ vdq                       4 PE matmuls, PSUM accum
  denom[h']   += e.T @ ones                      1 PE matmul
  out[h', :]  = o_num[h', :] / denom[h']         final normalize, bf16

The PE is the pacing engine (~1.8us/tile: QK 0.64 + PV/sums 1.14).  Both
sequences run as one flat 32-tile stream; all chunk DMAs are issued
upfront so the two HWDGE rings (K on sync, V on scalar) self-pace on
buffer release and never starve the PE.  QK+exp+dequant run two tiles
ahead of PV.  Expected rel err ~1.7e-2 (numpy-exact sim of this pipeline;
K-e3m4 rounding dominates).
"""

import numpy as np
import ml_dtypes

from concourse import bass, bacc, mybir, tile
from concourse.bass_utils import run_bass_kernel_spmd

# Problem constants (hardcoded per the grading contract).
B = 16          # total sequences
H = 16          # heads
D = 128         # head dim
BLOCK = 16      # tokens per cache block
BPS = 128       # blocks per sequence
NB = B * BPS    # total cache blocks
S = BPS * BLOCK # max context per sequence (2048)
SCALE = 0.08838834764831845

N_CORES = 8
B2 = B // N_CORES             # sequences per core (2)
T = S // 128                  # 128-token tiles per sequence (16)
CHUNKS = (1, 3, 4, 4, 3, 1)   # per-seq KV stream chunking (tiles per DMA)
assert sum(CHUNKS) == T
HD = H * D                    # 2048: free width of one V tile / K tile

F32 = mybir.dt.float32
BF16 = mybir.dt.bfloat16
E3 = mybir.dt.float8e3
I8 = mybir.dt.int8
NP_BF16 = ml_dtypes.bfloat16
NP_E3 = ml_dtypes.float8_e3m4

MASK_NEG = -30000.0  # exp(x + MASK_NEG) == 0 in fp32 for any |x| < ~100


def build_nc(b2=B2, chunks=CHUNKS):
    """Build the per-core Bass graph (SPMD: same graph on all 8 cores)."""
    t_tiles = sum(chunks)
    sizes = sorted(set(chunks))
    nc = bacc.Bacc("TRN2", target_bir_lowering=False, debug=False)

    n_of = {sz: sum(1 for c in chunks if c == sz) for sz in sizes}
    # K chunk: [d=128, (tile, h, s)] fp8e3; V chunk: [s=128, (tile, h, d)] int8
    kk_p = {sz: nc.declare_dram_parameter(
        f"kk{sz}", [b2, n_of[sz], 128, sz * HD], E3, isOutput=False)
        for sz in sizes}
    vv_p = {sz: nc.declare_dram_parameter(
        f"vv{sz}", [b2, n_of[sz], 128, sz * HD], I8, isOutput=False)
        for sz in sizes}
    qt = nc.declare_dram_parameter("qt", [b2, 128, H], BF16, isOutput=False)
    bias = nc.declare_dram_parameter("bias", [b2, 128, t_tiles], F32,
                                     isOutput=False)
    vsc = nc.declare_dram_parameter("vsc", [b2, 128, t_tiles], F32,
                                    isOutput=False)
    # PV numerator in all-heads layout [b, h', (h, d)], already normalized;
    # the host extracts the h'==h diagonal
    out = nc.declare_dram_parameter("out", [b2, H, HD], BF16, isOutput=True)

    # global chunk list across both sequences
    gchunks = []       # (b, sz, param_idx, first_local_tile)
    for b in range(b2):
        seen = {sz: 0 for sz in sizes}
        t0 = 0
        for sz in chunks:
            gchunks.append((b, sz, seen[sz], t0))
            seen[sz] += 1
            t0 += sz
    tile2chunk = []    # global tile -> global chunk
    for ci, (_, sz, _, _) in enumerate(gchunks):
        tile2chunk += [ci] * sz
    n_gtiles = b2 * t_tiles

    with tile.TileContext(nc) as tc:
        with (
            tc.tile_pool(name="const", bufs=1) as cpool,
            tc.tile_pool(name="kpool", bufs=2) as kpool,
            tc.tile_pool(name="vpool", bufs=2) as vpool,
            tc.tile_pool(name="vdq", bufs=4) as dqpool,
            tc.tile_pool(name="small", bufs=2) as spool,
            tc.tile_pool(name="pscore", bufs=3,
                         space=bass.MemorySpace.PSUM) as pscore,
            tc.tile_pool(name="pacc", bufs=1,
                         space=bass.MemorySpace.PSUM) as pacc,
        ):
            ones_t = cpool.tile([128, 1], BF16, tag="ones")
            nc.gpsimd.memset(ones_t[:], 1.0)

            # small per-seq inputs: ship before the chunk stream
            qt_sb, bias_sb, vsc_sb = {}, {}, {}
            for b in range(b2):
                qt_sb[b] = spool.tile([128, H], BF16, tag="qt_sb",
                                      name=f"qt_sb{b}")
                nc.scalar.dma_start(out=qt_sb[b][:], in_=qt[b])
                bias_sb[b] = spool.tile([128, t_tiles], F32, tag="bias_sb",
                                        name=f"bias_sb{b}")
                nc.scalar.dma_start(out=bias_sb[b][:], in_=bias[b])
                vsc_sb[b] = spool.tile([128, t_tiles], F32, tag="vsc_sb",
                                       name=f"vsc_sb{b}")
                nc.scalar.dma_start(out=vsc_sb[b][:], in_=vsc[b])

            hw = HD // 2
            # one accumulator set shared by both sequences (PSUM is 8 banks);
            # seq1's first accumulate waits on seq0's finalize reads
            ps_o_lo = pacc.tile([H, hw], F32, tag="ps_o_lo")
            ps_o_hi = pacc.tile([H, hw], F32, tag="ps_o_hi")
            ps_sums = pacc.tile([H, 1], F32, tag="ps_sums")

            kk_tiles, vv_tiles = {}, {}
            issued = [0]   # chunks issued so far

            def issue_up_to(ci_max):
                """Lazily issue chunk DMAs so ~2 chunks stay in flight."""
                while issued[0] <= min(ci_max, len(gchunks) - 1):
                    ci = issued[0]
                    b, sz, pi, _ = gchunks[ci]
                    nbuf = 3 if sz >= 3 else 2
                    kc = kpool.tile([128, sz * HD], E3, tag=f"kk{sz}",
                                    bufs=nbuf, name=f"kc{ci}")
                    nc.sync.dma_start(out=kc[:], in_=kk_p[sz][b, pi])
                    vc = vpool.tile([128, sz * HD], I8, tag=f"vv{sz}",
                                    bufs=nbuf, name=f"vc{ci}")
                    nc.scalar.dma_start(out=vc[:], in_=vv_p[sz][b, pi])
                    kk_tiles[ci], vv_tiles[ci] = kc, vc
                    issued[0] += 1

            def qk_exp(g):
                """QK (16 PE pairs) + exp -> e_t [128, H] bf16."""
                ci = tile2chunk[g]
                b, sz, _, ct0 = gchunks[ci]
                t = g - b * t_tiles               # local tile index
                kc = kk_tiles[ci]
                ps_sc = pscore.tile([128, H], F32, tag="ps_sc", bufs=3)
                base = (t - ct0) * HD
                for hh in range(H):
                    o0 = base + hh * 128
                    nc.tensor.matmul(
                        ps_sc[:, hh:hh + 1],
                        kc[:, o0:o0 + 128],
                        qt_sb[b][:, hh:hh + 1],
                        start=True, stop=True,
                        skip_group_check=True,
                    )
                e_t = spool.tile([128, H], BF16, tag="e_t", bufs=4)
                nc.scalar.activation(
                    e_t[:], ps_sc[:],
                    mybir.ActivationFunctionType.Exp,
                    bias=bias_sb[b][:, t:t + 1], scale=1.0,
                )
                return e_t

            def dequant(g):
                """V int8 -> bf16 with per-token scale (DVE)."""
                ci = tile2chunk[g]
                b, sz, _, ct0 = gchunks[ci]
                t = g - b * t_tiles
                vc = vv_tiles[ci]
                o0 = (t - ct0) * HD
                vq = dqpool.tile([128, HD], BF16, tag="vdq", bufs=4)
                nc.vector.tensor_scalar_mul(vq[:], vc[:, o0:o0 + HD],
                                            vsc_sb[b][:, t:t + 1])
                return vq

            def finalize(b):
                recip = spool.tile([H, 1], F32, tag="recip")
                nc.vector.reciprocal(recip[:], ps_sums[:])
                o_lo = spool.tile([H, hw], BF16, tag="o_lo")
                o_hi = spool.tile([H, hw], BF16, tag="o_hi")
                nc.scalar.mul(o_lo[:], ps_o_lo[:], recip[:])
                nc.vector.tensor_scalar_mul(o_hi[:], ps_o_hi[:], recip[:])
                if b == b2 - 1:
                    nc.sync.dma_start(out=out[b][:, 0:hw], in_=o_lo[:])
                    nc.scalar.dma_start(out=out[b][:, hw:], in_=o_hi[:])
                else:
                    nc.gpsimd.dma_start(out=out[b][:, 0:hw], in_=o_lo[:])
                    nc.gpsimd.dma_start(out=out[b][:, hw:], in_=o_hi[:])

            issue_up_to(1)
            pend = [(qk_exp(0), dequant(0))]
            issue_up_to(tile2chunk[1] + 2)
            pend.append((qk_exp(1), dequant(1)))
            for g in range(n_gtiles):
                if g + 2 < n_gtiles:
                    # keep ~2 chunks of DMA in flight past the one QK reads
                    issue_up_to(tile2chunk[g + 2] + 2)
                    pend.append((qk_exp(g + 2), dequant(g + 2)))
                e_t, vq = pend.pop(0)

                b = g // t_tiles
                t = g - b * t_tiles
                first = t == 0
                last = t == t_tiles - 1
                nc.tensor.matmul(ps_sums[:], e_t[:], ones_t[:],
                                 start=first, stop=last,
                                 skip_group_check=True)
                for n in range(4):
                    o0 = n * 512
                    dst = ps_o_lo if n < 2 else ps_o_hi
                    nc.tensor.matmul(
                        dst[:, (n % 2) * 512:(n % 2 + 1) * 512],
                        e_t[:],
                        vq[:, o0:o0 + 512],
                        start=first, stop=last,
                        skip_group_check=True,
                    )
                if last:
                    finalize(b)

    nc.compile()
    return nc


def prep_in_maps(q, k, v, k_cache, v_cache, block_tables, slot_mapping,
                 context_lens):
    """Host-side scatter + paged gather + 8-bit encode + per-core shards."""
    q = np.asarray(q, np.float32)
    k = np.asarray(k, np.float32)
    v = np.asarray(v, np.float32)
    k_cache = np.asarray(k_cache, np.float32)
    v_cache = np.asarray(v_cache, np.float32)
    block_tables = np.asarray(block_tables, np.int32)
    slot_mapping = np.asarray(slot_mapping, np.int64)
    context_lens = np.asarray(context_lens, np.int32)

    nb, block_size, h, d = k_cache.shape
    kc = k_cache.reshape(nb * block_size, h, d).copy()
    kc[slot_mapping] = k
    vc = v_cache.reshape(nb * block_size, h, d).copy()
    vc[slot_mapping] = v
    k_seq = kc.reshape(nb, block_size, h, d)[block_tables].reshape(B, S, h, d)
    v_seq = vc.reshape(nb, block_size, h, d)[block_tables].reshape(B, S, h, d)

    # K: [B,S,H,D] -> [B, D, T, H, 128s] fp8 e3m4
    kt = np.ascontiguousarray(
        k_seq.reshape(B, T, 128, H, D).transpose(0, 4, 1, 3, 2)
    ).astype(NP_E3).reshape(B, 128, T * HD)

    # V: int8 with per-token scale. st: [B, S]
    st = np.abs(v_seq).reshape(B, S, HD).max(axis=2) / 127.0
    v8 = np.clip(np.round(v_seq.reshape(B, S, HD) / st[:, :, None]),
                 -127, 127).astype(np.int8)
    # [B, S, HD] -> [B, 128s, T, HD]
    v8t = np.ascontiguousarray(
        v8.reshape(B, T, 128, HD).transpose(0, 2, 1, 3))

    sizes = sorted(set(CHUNKS))
    kk_parts = {sz: [] for sz in sizes}
    v_parts = {sz: [] for sz in sizes}
    t0 = 0
    for sz in CHUNKS:
        kk_parts[sz].append(
            kt[:, None, :, t0 * HD:(t0 + sz) * HD])
        v_parts[sz].append(np.ascontiguousarray(
            v8t[:, :, t0:t0 + sz]).reshape(B, 1, 128, sz * HD))
        t0 += sz
    kk_host = {sz: np.ascontiguousarray(np.concatenate(kk_parts[sz], axis=1))
               for sz in sizes}
    v_host = {sz: np.concatenate(v_parts[sz], axis=1) for sz in sizes}

    qs = (q * SCALE).astype(NP_BF16)
    qt_host = np.ascontiguousarray(qs.transpose(0, 2, 1))  # [B, D, H]
    s_idx = np.arange(S, dtype=np.int64)
    m = np.where(s_idx[None, :] < context_lens[:, None].astype(np.int64),
                 0.0, MASK_NEG).astype(np.float32)
    bias_host = np.ascontiguousarray(m.reshape(B, T, 128).transpose(0, 2, 1))
    vsc_host = np.ascontiguousarray(
        st.astype(np.float32).reshape(B, T, 128).transpose(0, 2, 1))

    in_maps = []
    for i in range(N_CORES):
        lo, hi = i * B2, (i + 1) * B2
        im = {"qt": np.ascontiguousarray(qt_host[lo:hi]),
              "bias": np.ascontiguousarray(bias_host[lo:hi]),
              "vsc": np.ascontiguousarray(vsc_host[lo:hi])}
        for sz in sizes:
            im[f"kk{sz}"] = np.ascontiguousarray(kk_host[sz][lo:hi])
            im[f"vv{sz}"] = np.ascontiguousarray(v_host[sz][lo:hi])
        in_maps.append(im)
    return in_maps


_NC = None


def _get_nc():
    global _NC
    if _NC is None:
        _NC = build_nc()
    return _NC


def run(inputs, trace=False, **spmd_kwargs):
    """Run on hardware; returns (full_output, BassKernelResults)."""
    nc = _get_nc()
    in_maps = prep_in_maps(**inputs)
    res = run_bass_kernel_spmd(nc, in_maps, core_ids=list(range(N_CORES)),
                               trace=trace, **spmd_kwargs)
    out_full = np.concatenate([res.results[i]["out"] for i in range(N_CORES)],
                              axis=0).astype(np.float32)
    # extract the h'==h diagonal: [B, H, H*D] -> [B, H, D]
    hh = np.arange(H)
    out = out_full.reshape(B, H, H, D)[:, hh, hh, :]
    return np.ascontiguousarray(out), res


def kernel(**inputs) -> np.ndarray:
    out, _ = run(inputs, trace=False)
    return out


# revision 15
# speedup vs baseline: 1.1758x; 1.0853x over previous
"""Paged-attention decode kernel for 8 TRN2 NeuronCores (8-bit KV stream).

Problem: B=16 decode sequences, H=16 heads, D=128 head dim, paged KV cache
(2048 blocks x 16 tokens), context S=2048 per sequence.

Sharding: data-parallel over sequences -- 2 sequences per core, no
collectives.  The host applies the KV-cache scatter (slot_mapping), the
paged gather (block_tables), and 8-bit encoding while laying out per-core
shards; the device does the full masked single-token attention.

8-bit wire format (halves the 33.6MB/core bf16 KV stream to 16.8MB):
  K: float8 e3m4, consumed DIRECTLY by the PE as the stationary QK operand
     (mixed fp8xbf16 matmul decodes exactly, subnormals included).
  V: int8 with a per-token fp32 scale, dequantized to bf16 on the DVE
     (tensor_scalar hits 2X mode even with int8 input: ~1.28us per tile),
     then consumed as the moving PV operand.

Device math per (seq, 128-token tile), fp32 accumulate:
  scores[s,h] = sum_d K8[d,s] * (q[d,h]*SCALE)   16 PE pairs (~40ns each)
  e[s,h]      = exp(scores + mask_bias[s])       ScalarE, bias trick
  vdq[s,:]    = bf16(V8[s,:] * vscale[s])        DVE
  o_num[h',:] += e.T @ vdq                       4 PE matmuls, PSUM accum
  denom[h']   += e.T @ ones                      1 PE matmul
  out[h', :]  = o_num[h', :] / denom[h']         final normalize, bf16

The PE is the pacing engine (~1.8us/tile: QK 0.64 + PV/sums 1.14).  Both
sequences run as one flat 32-tile stream; all chunk DMAs are issued
upfront so the two HWDGE rings (K on sync, V on scalar) self-pace on
buffer release and never starve the PE.  QK+exp+dequant run two tiles
ahead of PV.  Expected rel err ~1.7e-2 (numpy-exact sim of this pipeline;
K-e3m4 rounding dominates).
"""

import numpy as np
import ml_dtypes

from concourse import bass, bacc, mybir, tile
from concourse.bass_utils import run_bass_kernel_spmd

# Problem constants (hardcoded per the grading contract).
B = 16          # total sequences
H = 16          # heads
D = 128         # head dim
BLOCK = 16      # tokens per cache block
BPS = 128       # blocks per sequence
NB = B * BPS    # total cache blocks
S = BPS * BLOCK # max context per sequence (2048)
SCALE = 0.08838834764831845

N_CORES = 8
B2 = B // N_CORES             # sequences per core (2)
T = S // 128                  # 128-token tiles per sequence (16)
CHUNKS = (1, 3, 4, 4, 3, 1)   # per-seq KV stream chunking (tiles per DMA)
assert sum(CHUNKS) == T
HD = H * D                    # 2048: free width of one V tile / K tile

F32 = mybir.dt.float32
BF16 = mybir.dt.bfloat16
E3 = mybir.dt.float8e3
I8 = mybir.dt.int8
NP_BF16 = ml_dtypes.bfloat16
NP_E3 = ml_dtypes.float8_e3m4

MASK_NEG = -30000.0  # exp(x + MASK_NEG) == 0 in fp32 for any |x| < ~100


def build_nc(b2=B2, chunks=CHUNKS):
    """Build the per-core Bass graph (SPMD: same graph on all 8 cores)."""
    t_tiles = sum(chunks)
    sizes = sorted(set(chunks))
    nc = bacc.Bacc("TRN2", target_bir_lowering=False, debug=False)

    n_of = {sz: sum(1 for c in chunks if c == sz) for sz in sizes}
    # K chunk: [d=128, (tile, h, s)] fp8e3; V chunk: [s=128, (tile, h, d)] int8
    kk_p = {sz: nc.declare_dram_parameter(
        f"kk{sz}", [b2, n_of[sz], 128, sz * HD], E3, isOutput=False)
        for sz in sizes}
    vv_p = {sz: nc.declare_dram_parameter(
        f"vv{sz}", [b2, n_of[sz], 128, sz * HD], I8, isOutput=False)
        for sz in sizes}
    qt = nc.declare_dram_parameter("qt", [b2, 128, H], BF16, isOutput=False)
    bias = nc.declare_dram_parameter("bias", [b2, 128, t_tiles], F32,
                                     isOutput=False)
    vsc = nc.declare_dram_parameter("vsc", [b2, 128, t_tiles], F32,
                                    isOutput=False)
    # PV numerator in all-heads layout [b, h', (h, d)], already normalized;
    # the host extracts the h'==h diagonal
    out = nc.declare_dram_parameter("out", [b2, H, HD], BF16, isOutput=True)

    # global chunk list across both sequences
    gchunks = []       # (b, sz, param_idx, first_local_tile)
    for b in range(b2):
        seen = {sz: 0 for sz in sizes}
        t0 = 0
        for sz in chunks:
            gchunks.append((b, sz, seen[sz], t0))
            seen[sz] += 1
            t0 += sz
    tile2chunk = []    # global tile -> global chunk
    for ci, (_, sz, _, _) in enumerate(gchunks):
        tile2chunk += [ci] * sz
    n_gtiles = b2 * t_tiles

    with tile.TileContext(nc) as tc:
        with (
            tc.tile_pool(name="const", bufs=1) as cpool,
            tc.tile_pool(name="kpool", bufs=2) as kpool,
            tc.tile_pool(name="vpool", bufs=2) as vpool,
            tc.tile_pool(name="vdq", bufs=4) as dqpool,
            tc.tile_pool(name="small", bufs=2) as spool,
            tc.tile_pool(name="pscore", bufs=3,
                         space=bass.MemorySpace.PSUM) as pscore,
            tc.tile_pool(name="pacc", bufs=1,
                         space=bass.MemorySpace.PSUM) as pacc,
        ):
            ones_t = cpool.tile([128, 1], BF16, tag="ones")
            nc.gpsimd.memset(ones_t[:], 1.0)

            # small per-seq inputs: ship before the chunk stream
            qt_sb, bias_sb, vsc_sb = {}, {}, {}
            for b in range(b2):
                qt_sb[b] = spool.tile([128, H], BF16, tag="qt_sb",
                                      name=f"qt_sb{b}")
                nc.scalar.dma_start(out=qt_sb[b][:], in_=qt[b])
                bias_sb[b] = spool.tile([128, t_tiles], F32, tag="bias_sb",
                                        name=f"bias_sb{b}")
                nc.scalar.dma_start(out=bias_sb[b][:], in_=bias[b])
                vsc_sb[b] = spool.tile([128, t_tiles], F32, tag="vsc_sb",
                                       name=f"vsc_sb{b}")
                nc.scalar.dma_start(out=vsc_sb[b][:], in_=vsc[b])

            hw = HD // 2
            # one accumulator set shared by both sequences (PSUM is 8 banks);
            # seq1's first accumulate waits on seq0's finalize reads
            ps_o_lo = pacc.tile([H, hw], F32, tag="ps_o_lo")
            ps_o_hi = pacc.tile([H, hw], F32, tag="ps_o_hi")
            ps_sums = pacc.tile([H, 1], F32, tag="ps_sums")

            kk_tiles, vv_tiles = {}, {}
            issued = [0]   # chunks issued so far

            def issue_up_to(ci_max):
                """Lazily issue chunk DMAs so ~2 chunks stay in flight."""
                while issued[0] <= min(ci_max, len(gchunks) - 1):
                    ci = issued[0]
                    b, sz, pi, _ = gchunks[ci]
                    nbuf = 4 if sz == 4 else (3 if sz == 3 else 2)
                    kc = kpool.tile([128, sz * HD], E3, tag=f"kk{sz}",
                                    bufs=nbuf, name=f"kc{ci}")
                    nc.sync.dma_start(out=kc[:], in_=kk_p[sz][b, pi])
                    vc = vpool.tile([128, sz * HD], I8, tag=f"vv{sz}",
                                    bufs=nbuf, name=f"vc{ci}")
                    nc.scalar.dma_start(out=vc[:], in_=vv_p[sz][b, pi])
                    kk_tiles[ci], vv_tiles[ci] = kc, vc
                    issued[0] += 1

            def qk_exp(g):
                """QK (16 PE pairs) + exp -> e_t [128, H] bf16."""
                ci = tile2chunk[g]
                b, sz, _, ct0 = gchunks[ci]
                t = g - b * t_tiles               # local tile index
                kc = kk_tiles[ci]
                ps_sc = pscore.tile([128, H], F32, tag="ps_sc", bufs=3)
                base = (t - ct0) * HD
                for hh in range(H):
                    o0 = base + hh * 128
                    nc.tensor.matmul(
                        ps_sc[:, hh:hh + 1],
                        kc[:, o0:o0 + 128],
                        qt_sb[b][:, hh:hh + 1],
                        start=True, stop=True,
                        skip_group_check=True,
                    )
                e_t = spool.tile([128, H], BF16, tag="e_t", bufs=4)
                nc.scalar.activation(
                    e_t[:], ps_sc[:],
                    mybir.ActivationFunctionType.Exp,
                    bias=bias_sb[b][:, t:t + 1], scale=1.0,
                )
                return e_t

            def dequant(g):
                """V int8 -> bf16 with per-token scale (DVE)."""
                ci = tile2chunk[g]
                b, sz, _, ct0 = gchunks[ci]
                t = g - b * t_tiles
                vc = vv_tiles[ci]
                o0 = (t - ct0) * HD
                vq = dqpool.tile([128, HD], BF16, tag="vdq", bufs=4)
                nc.vector.tensor_scalar_mul(vq[:], vc[:, o0:o0 + HD],
                                            vsc_sb[b][:, t:t + 1])
                return vq

            def finalize(b):
                recip = spool.tile([H, 1], F32, tag="recip")
                nc.vector.reciprocal(recip[:], ps_sums[:])
                o_lo = spool.tile([H, hw], BF16, tag="o_lo")
                o_hi = spool.tile([H, hw], BF16, tag="o_hi")
                nc.scalar.mul(o_lo[:], ps_o_lo[:], recip[:])
                nc.vector.tensor_scalar_mul(o_hi[:], ps_o_hi[:], recip[:])
                if b == b2 - 1:
                    nc.sync.dma_start(out=out[b][:, 0:hw], in_=o_lo[:])
                    nc.scalar.dma_start(out=out[b][:, hw:], in_=o_hi[:])
                else:
                    nc.gpsimd.dma_start(out=out[b][:, 0:hw], in_=o_lo[:])
                    nc.gpsimd.dma_start(out=out[b][:, hw:], in_=o_hi[:])

            issue_up_to(1)
            pend = [(qk_exp(0), dequant(0))]
            issue_up_to(tile2chunk[1] + 2)
            pend.append((qk_exp(1), dequant(1)))
            for g in range(n_gtiles):
                if g + 2 < n_gtiles:
                    # keep ~2 chunks of DMA in flight past the one QK reads
                    issue_up_to(tile2chunk[g + 2] + 2)
                    pend.append((qk_exp(g + 2), dequant(g + 2)))
                e_t, vq = pend.pop(0)

                b = g // t_tiles
                t = g - b * t_tiles
                first = t == 0
                last = t == t_tiles - 1
                nc.tensor.matmul(ps_sums[:], e_t[:], ones_t[:],
                                 start=first, stop=last,
                                 skip_group_check=True)
                for n in range(4):
                    o0 = n * 512
                    dst = ps_o_lo if n < 2 else ps_o_hi
                    nc.tensor.matmul(
                        dst[:, (n % 2) * 512:(n % 2 + 1) * 512],
                        e_t[:],
                        vq[:, o0:o0 + 512],
                        start=first, stop=last,
                        skip_group_check=True,
                    )
                if last:
                    finalize(b)

    nc.compile()
    return nc


def prep_in_maps(q, k, v, k_cache, v_cache, block_tables, slot_mapping,
                 context_lens):
    """Host-side scatter + paged gather + 8-bit encode + per-core shards."""
    q = np.asarray(q, np.float32)
    k = np.asarray(k, np.float32)
    v = np.asarray(v, np.float32)
    k_cache = np.asarray(k_cache, np.float32)
    v_cache = np.asarray(v_cache, np.float32)
    block_tables = np.asarray(block_tables, np.int32)
    slot_mapping = np.asarray(slot_mapping, np.int64)
    context_lens = np.asarray(context_lens, np.int32)

    nb, block_size, h, d = k_cache.shape
    kc = k_cache.reshape(nb * block_size, h, d).copy()
    kc[slot_mapping] = k
    vc = v_cache.reshape(nb * block_size, h, d).copy()
    vc[slot_mapping] = v
    k_seq = kc.reshape(nb, block_size, h, d)[block_tables].reshape(B, S, h, d)
    v_seq = vc.reshape(nb, block_size, h, d)[block_tables].reshape(B, S, h, d)

    # K: [B,S,H,D] -> [B, D, T, H, 128s] fp8 e3m4
    kt = np.ascontiguousarray(
        k_seq.reshape(B, T, 128, H, D).transpose(0, 4, 1, 3, 2)
    ).astype(NP_E3).reshape(B, 128, T * HD)

    # V: int8 with per-token scale. st: [B, S]
    st = np.abs(v_seq).reshape(B, S, HD).max(axis=2) / 127.0
    v8 = np.clip(np.round(v_seq.reshape(B, S, HD) / st[:, :, None]),
                 -127, 127).astype(np.int8)
    # [B, S, HD] -> [B, 128s, T, HD]
    v8t = np.ascontiguousarray(
        v8.reshape(B, T, 128, HD).transpose(0, 2, 1, 3))

    sizes = sorted(set(CHUNKS))
    kk_parts = {sz: [] for sz in sizes}
    v_parts = {sz: [] for sz in sizes}
    t0 = 0
    for sz in CHUNKS:
        kk_parts[sz].append(
            kt[:, None, :, t0 * HD:(t0 + sz) * HD])
        v_parts[sz].append(np.ascontiguousarray(
            v8t[:, :, t0:t0 + sz]).reshape(B, 1, 128, sz * HD))
        t0 += sz
    kk_host = {sz: np.ascontiguousarray(np.concatenate(kk_parts[sz], axis=1))
               for sz in sizes}
    v_host = {sz: np.concatenate(v_parts[sz], axis=1) for sz in sizes}

    qs = (q * SCALE).astype(NP_BF16)
    qt_host = np.ascontiguousarray(qs.transpose(0, 2, 1))  # [B, D, H]
    s_idx = np.arange(S, dtype=np.int64)
    m = np.where(s_idx[None, :] < context_lens[:, None].astype(np.int64),
                 0.0, MASK_NEG).astype(np.float32)
    bias_host = np.ascontiguousarray(m.reshape(B, T, 128).transpose(0, 2, 1))
    vsc_host = np.ascontiguousarray(
        st.astype(np.float32).reshape(B, T, 128).transpose(0, 2, 1))

    in_maps = []
    for i in range(N_CORES):
        lo, hi = i * B2, (i + 1) * B2
        im = {"qt": np.ascontiguousarray(qt_host[lo:hi]),
              "bias": np.ascontiguousarray(bias_host[lo:hi]),
              "vsc": np.ascontiguousarray(vsc_host[lo:hi])}
        for sz in sizes:
            im[f"kk{sz}"] = np.ascontiguousarray(kk_host[sz][lo:hi])
            im[f"vv{sz}"] = np.ascontiguousarray(v_host[sz][lo:hi])
        in_maps.append(im)
    return in_maps


_NC = None


def _get_nc():
    global _NC
    if _NC is None:
        _NC = build_nc()
    return _NC


def run(inputs, trace=False, **spmd_kwargs):
    """Run on hardware; returns (full_output, BassKernelResults)."""
    nc = _get_nc()
    in_maps = prep_in_maps(**inputs)
    res = run_bass_kernel_spmd(nc, in_maps, core_ids=list(range(N_CORES)),
                               trace=trace, **spmd_kwargs)
    out_full = np.concatenate([res.results[i]["out"] for i in range(N_CORES)],
                              axis=0).astype(np.float32)
    # extract the h'==h diagonal: [B, H, H*D] -> [B, H, D]
    hh = np.arange(H)
    out = out_full.reshape(B, H, H, D)[:, hh, hh, :]
    return np.ascontiguousarray(out), res


def kernel(**inputs) -> np.ndarray:
    out, _ = run(inputs, trace=False)
    return out
